# revision 1
# baseline (speedup 1.0000x reference)
"""Trainium2 Bass kernel for nn_FeatureGenKerasV2.

Contract: kernel(x) with x [100000, 115, 3] f32 -> [1, 200, 1198] f32.

Reference semantics:
  - global: cond = (count_nonzero(x[:,40:61]) > count_nonzero(x[:,94:115]))
  - per frame t<200: features built from hand(sel by cond)/pose/lip coords,
    temporal diff vs frame t+1, static-pair distances, hand mask.

Sharding (8 cores, embarrassingly parallel):
  - count phase: core c counts nonzeros of both hand regions over frames
    [12500c, 12500(c+1)) and outputs the scalar partial (cntL - cntR).
  - feature phase: core c computes BOTH left/right feature variants for its
    output frames [25c, 25c+26) (1-frame halo sliced host-side) and writes
    yl_c/yr_c [25, 1198].
  - unshard: the host sums the 8 exact integer-valued partials, picks the
    variant (cond = diff > 0), and concatenates the per-core slices.
"""

import numpy as np

import concourse.bass as bass
import concourse.tile as tile
from concourse import bacc, mybir
from concourse import bass_utils

F32 = mybir.dt.float32
ALU = mybir.AluOpType
ACTF = mybir.ActivationFunctionType

NCORES = 8
T_TOT = 100000
SHARD = T_TOT // NCORES          # 12500 count frames per core
P = 125                          # SBUF partitions used for counting
FPP = SHARD // P                 # 100 frames per partition
NCHUNK = 20                      # count chunks
FPC = FPP // NCHUNK              # 10 frames (per partition) per chunk
OUTF = 25                        # output frames per core
BF = OUTF + 1                    # feature frames per core (1 halo)

# static pair index tables (match np.triu_indices order used by reference)
_HIU = np.triu_indices(21, 1)    # 210 hand pairs
_PIU = np.triu_indices(25, 1)    # 300 pose pairs
_LIU = np.triu_indices(20, 1)    # 190 lip pairs
NH, NP_, NL = 210, 300, 190


def _pairmat(nj, iu):
    g = np.zeros((nj, len(iu[0])), np.float32)
    g[iu[0], np.arange(len(iu[0]))] = 1.0
    g[iu[1], np.arange(len(iu[1]))] -= 1.0
    return g


def build_bass():
    nc = bacc.Bacc("TRN2", target_bir_lowering=False, debug=False,
                   num_devices=NCORES)

    xs = nc.dram_tensor("xs", [SHARD, 345], F32, kind="ExternalInput")
    xb = nc.dram_tensor("xb", [BF, 115, 3], F32, kind="ExternalInput")
    # per-region joint-major layout: 5 regions x 3 coords x BF frames,
    # regions: handL, handR, pose, lip1, lip2 (each region's joints at
    # partition 0 so PE matmul base-partition rules are satisfied)
    xreg = nc.dram_tensor("xreg", [25, 5 * 3 * BF], F32, kind="ExternalInput")
    gh_d = nc.dram_tensor("gh", [21, NH], F32, kind="ExternalInput")
    gp_d = nc.dram_tensor("gp", [25, NP_], F32, kind="ExternalInput")
    gl_d = nc.dram_tensor("gl", [20, NL], F32, kind="ExternalInput")
    yl = nc.dram_tensor("yl", [OUTF, 1198], F32, kind="ExternalOutput")
    yr = nc.dram_tensor("yr", [OUTF, 1198], F32, kind="ExternalOutput")
    pdif = nc.dram_tensor("pdif", [1, 1], F32, kind="ExternalOutput")

    with tile.TileContext(nc) as tc:
        with (
            tc.tile_pool(name="cnt_in", bufs=10) as cnt_in,
            tc.tile_pool(name="cnt_scr", bufs=8) as cnt_scr,
            tc.tile_pool(name="persist", bufs=1) as persist,
            tc.tile_pool(name="fb", bufs=1) as fb,
            tc.tile_pool(name="psum", bufs=2, space=bass.MemorySpace.PSUM) as psum,
            tc.tile_pool(name="psum1", bufs=1, space=bass.MemorySpace.PSUM) as psum1,
        ):
            # ---------------- feature phase (both variants) ----------------
            XB = fb.tile([BF, 115, 3], F32)
            nc.sync.dma_start(XB[:], xb[:])
            XR = fb.tile([25, 5 * 3 * BF], F32)
            nc.sync.dma_start(XR[:], xreg[:])
            gh = fb.tile([21, NH], F32)
            nc.sync.dma_start(gh[:], gh_d[:])
            gp = fb.tile([25, NP_], F32)
            nc.sync.dma_start(gp[:], gp_d[:])
            gl = fb.tile([20, NL], F32)
            nc.sync.dma_start(gl[:], gl_d[:])

            # shifted copy (frame t+1) for temporal diff
            XBs = fb.tile([OUTF, 115, 3], F32)
            nc.sync.dma_start(XBs[:], XB[1:BF, :, :])
            D = fb.tile([OUTF, 115, 3], F32)
            nc.vector.tensor_sub(D[:], XB[0:OUTF, :, :], XBs[:])

            # mirrored-left hand coords (x negated), plain and temporal-diff
            tmpL = fb.tile([BF, 21, 3], F32)
            nc.scalar.mul(tmpL[:, :, 0:1], XB[:, 40:61, 0:1], -1.0)
            nc.scalar.copy(tmpL[:, :, 1:3], XB[:, 40:61, 1:3])
            tmpDL = fb.tile([OUTF, 21, 3], F32)
            nc.scalar.mul(tmpDL[:, :, 0:1], D[:, 40:61, 0:1], -1.0)
            nc.scalar.copy(tmpDL[:, :, 1:3], D[:, 40:61, 1:3])

            # pairwise squared distances via PE: diff_c = Xreg_c.T @ G
            def dist2(dst, region, nj, gt, npair, ncoord):
                for c in range(ncoord):
                    pdsq = psum.tile([BF, npair], F32, tag="pdif")
                    base = region * 3 * BF + c * BF
                    nc.tensor.matmul(
                        pdsq[:], XR[0:nj, base:base + BF], gt[:])
                    if c == 0:
                        nc.scalar.square(dst[:], pdsq[:])
                    else:
                        sq = fb.tile([BF, npair], F32, tag="sqt")
                        nc.scalar.square(sq[:], pdsq[:])
                        nc.vector.tensor_add(dst[:], dst[:], sq[:])

            hd2L = fb.tile([BF, NH], F32)
            dist2(hd2L, 0, 21, gh, NH, 3)
            hd2R = fb.tile([BF, NH], F32)
            dist2(hd2R, 1, 21, gh, NH, 3)
            pd2 = fb.tile([BF, NP_], F32)
            dist2(pd2, 2, 25, gp, NP_, 2)
            ol2 = fb.tile([BF, NL], F32)
            dist2(ol2, 3, 20, gl, NL, 2)
            il2 = fb.tile([BF, NL], F32)
            dist2(il2, 4, 20, gl, NL, 2)

            # hand masks
            sumL = fb.tile([BF, 1], F32)
            nc.vector.reduce_sum(out=sumL[:], in_=XB[:, 40:61, :],
                                 axis=mybir.AxisListType.XY)
            sumR = fb.tile([BF, 1], F32)
            nc.vector.reduce_sum(out=sumR[:], in_=XB[:, 94:115, :],
                                 axis=mybir.AxisListType.XY)
            maskL = fb.tile([BF, 1], F32)
            nc.vector.tensor_scalar(out=maskL[:], in0=sumL[:], scalar1=0.0,
                                    scalar2=None, op0=ALU.not_equal)
            maskR = fb.tile([BF, 1], F32)
            nc.vector.tensor_scalar(out=maskR[:], in0=sumR[:], scalar1=0.0,
                                    scalar2=None, op0=ALU.not_equal)

            FEATL = fb.tile([OUTF, 1198], F32)
            FEATR = fb.tile([OUTF, 1198], F32)

            def v3(ft, lo, hi):
                return ft[:, lo:hi].rearrange("p (j c) -> p j c", c=3)

            def v2(ft, lo, hi):
                return ft[:, lo:hi].rearrange("p (j c) -> p j c", c=2)

            for FT, hnd, dhnd, hd2, msk in (
                    (FEATR, XB[0:OUTF, 94:115, :], D[:, 94:115, :],
                     hd2R, maskR),
                    (FEATL, tmpL[0:OUTF, :, :], tmpDL[:], hd2L, maskL)):
                nc.scalar.copy(v3(FT, 0, 63), hnd)
                nc.scalar.copy(v2(FT, 63, 113), XB[0:OUTF, 61:86, 0:2])
                nc.scalar.copy(v2(FT, 113, 153), XB[0:OUTF, 0:20, 0:2])
                nc.scalar.copy(v3(FT, 153, 216), dhnd)
                nc.scalar.copy(v2(FT, 216, 266), D[:, 61:86, 0:2])
                nc.scalar.copy(v2(FT, 266, 306), D[:, 0:20, 0:2])
                nc.scalar.sqrt(FT[:, 306:516], hd2[0:OUTF, :])
                nc.vector.tensor_copy(FT[:, 1196:1197], msk[0:OUTF, :])
                nc.vector.tensor_scalar(
                    out=FT[:, 1197:1198], in0=msk[0:OUTF, :],
                    scalar1=1.0, scalar2=None, op0=ALU.add)

            # cond-invariant distance block: compute once, copy across
            nc.scalar.sqrt(FEATR[:, 516:816], pd2[0:OUTF, :])
            nc.scalar.sqrt(FEATR[:, 816:1006], ol2[0:OUTF, :])
            nc.scalar.sqrt(FEATR[:, 1006:1196], il2[0:OUTF, :])
            nc.scalar.copy(FEATL[:, 516:1196], FEATR[:, 516:1196])

            # mirror x coords of pose/lip blocks in the left variant
            for (lo, hi) in ((63, 113), (113, 153), (216, 266), (266, 306)):
                vv = v2(FEATL, lo, hi)
                nc.vector.tensor_scalar(
                    out=vv[:, :, 0:1], in0=vv[:, :, 0:1], scalar1=-1.0,
                    scalar2=None, op0=ALU.mult)

            nc.sync.dma_start(yr[:], FEATR[:])
            nc.sync.dma_start(yl[:], FEATL[:])

            # ---------------- count phase ----------------
            # one contiguous 225-elem span per frame (lefth|junk|righth) ->
            # 900B DMA descriptors at near line rate on the SWDGE path; the
            # fused not_equal+accum DVE op runs only over the two 63-elem
            # hand slices (strided, 1x) into per-partition accumulators.
            xsr = xs[:].rearrange("(p f) c -> p f c", p=P)  # [125,100,345]
            BF16 = mybir.dt.bfloat16
            SPW = 225
            acc = persist.tile([P, 2 * NCHUNK], F32)
            for k in range(NCHUNK):
                ts_ = cnt_in.tile([P, FPC, SPW], F32, tag="cin")
                sl = slice(k * FPC, (k + 1) * FPC)
                nc.gpsimd.dma_start(ts_[:], xsr[:, sl, 120:345])
                for h, (lo, hi) in enumerate(((0, 63), (162, 225))):
                    scr = cnt_scr.tile([P, FPC, 63], BF16, tag="scr")
                    nc.vector.tensor_scalar(
                        out=scr[:], in0=ts_[:, :, lo:hi],
                        scalar1=0.0, scalar2=None, op0=ALU.not_equal,
                        op1=ALU.add,
                        accum_out=acc[:, h * NCHUNK + k:h * NCHUNK + k + 1])

            red = persist.tile([P, 2], F32)
            nc.vector.reduce_sum(out=red[:, 0:1], in_=acc[:, 0:NCHUNK],
                                 axis=mybir.AxisListType.X)
            nc.vector.reduce_sum(out=red[:, 1:2], in_=acc[:, NCHUNK:2 * NCHUNK],
                                 axis=mybir.AxisListType.X)
            dif = persist.tile([P, 1], F32)
            nc.vector.tensor_sub(dif[:], red[:, 0:1], red[:, 1:2])
            onesf = persist.tile([P, 1], F32)
            nc.vector.memset(onesf[:], 1.0)
            pd_sc = psum1.tile([1, 1], F32)
            nc.tensor.matmul(pd_sc[:], dif[:], onesf[:])
            sdif = persist.tile([1, 1], F32)
            nc.scalar.copy(sdif[:], pd_sc[:])

            nc.sync.dma_start(pdif[:], sdif[:])

    nc.compile()
    return nc


_NC_CACHE = None


def _get_nc():
    global _NC_CACHE
    if _NC_CACHE is None:
        _NC_CACHE = build_bass()
    return _NC_CACHE


def make_in_maps(x: np.ndarray):
    x = np.ascontiguousarray(np.asarray(x, dtype=np.float32))
    assert x.shape == (T_TOT, 115, 3)
    xf = x.reshape(T_TOT, 345)
    gh = _pairmat(21, _HIU)
    gp = _pairmat(25, _PIU)
    gl = _pairmat(20, _LIU)
    in_maps = []
    regions = ((40, 61), (94, 115), (61, 86), (0, 20), (20, 40))
    for c in range(NCORES):
        xs = xf[c * SHARD:(c + 1) * SHARD]
        xb = x[c * OUTF:c * OUTF + BF]                      # [26,115,3]
        xreg = np.zeros((25, 5 * 3 * BF), np.float32)
        for r, (j0, j1) in enumerate(regions):
            blk = xb[:, j0:j1, :].transpose(1, 2, 0)        # [J,3,BF]
            xreg[0:j1 - j0, r * 3 * BF:(r + 1) * 3 * BF] = \
                blk.reshape(j1 - j0, 3 * BF)
        in_maps.append({
            "xs": xs, "xb": np.ascontiguousarray(xb), "xreg": xreg,
            "gh": gh, "gp": gp, "gl": gl,
        })
    return in_maps


def run_device(x: np.ndarray, **kw):
    nc = _get_nc()
    in_maps = make_in_maps(x)
    res = bass_utils.run_bass_kernel_spmd(
        nc, in_maps, core_ids=list(range(NCORES)), **kw)
    # global left/right decision from the 8 exact integer-valued partials
    diff = np.float32(sum(np.float32(r["pdif"][0, 0]) for r in res.results))
    key = "yl" if diff > 0 else "yr"
    out = np.concatenate([r[key] for r in res.results], axis=0)
    return out.reshape(1, 200, 1198).astype(np.float32, copy=False), res


def kernel(x: np.ndarray) -> np.ndarray:
    return run_device(x)[0]


if __name__ == "__main__":
    rng = np.random.default_rng(0)
    x = rng.standard_normal((T_TOT, 115, 3), dtype=np.float32)
    out = kernel(x)
    print(out.shape, out.dtype, float(np.linalg.norm(out)))



# revision 3
# speedup vs baseline: 2.0773x; 2.0773x over previous
"""Trainium2 Bass kernel for nn_FeatureGenKerasV2.

Contract: kernel(x) with x [100000, 115, 3] f32 -> [1, 200, 1198] f32.

Reference semantics:
  - global: cond = (count_nonzero(x[:,40:61]) > count_nonzero(x[:,94:115]))
  - per frame t<200: features built from hand(sel by cond)/pose/lip coords,
    temporal diff vs frame t+1, static-pair distances, hand mask.

Sharding (8 cores, embarrassingly parallel over frames):
  - count phase: core c counts nonzeros of both hand regions over frames
    [12500c, 12500(c+1)). The hand columns are sliced out host-side into
    contiguous per-core [12500, 63] arrays (bf16: exactly count-preserving
    for |x| >= 2^-133, i.e. any normal input) so the device streams them
    at DMA line rate instead of 900B strided descriptors. The per-partition
    per-chunk partial sums [125, 2*NCHUNK] are returned raw and the host
    performs the exact scalar all-reduce (integer-valued f32, summed in f64).
  - feature phase: core c computes BOTH left/right feature variants for its
    output frames [25c, 25c+25) (1-frame halo sliced host-side) and writes
    yl_c/yr_c [25, 1198].
  - unshard: host sums the partials, picks the variant (cond = diff > 0),
    and concatenates the per-core slices.
"""

import numpy as np
import ml_dtypes

import concourse.bass as bass
import concourse.tile as tile
from concourse import bacc, mybir
from concourse import bass_utils

F32 = mybir.dt.float32
BF16 = mybir.dt.bfloat16
ALU = mybir.AluOpType

NCORES = 8
T_TOT = 100000
SHARD = T_TOT // NCORES          # 12500 count frames per core
P = 125                          # SBUF partitions used for counting
FPP = SHARD // P                 # 100 frames per partition
HW = 63                          # hand elements per frame
CPP = FPP * HW                   # 6300 hand elems per partition
NCHUNK = 5                       # count chunks per hand
CW = CPP // NCHUNK               # 1260 elems per partition per chunk
OUTF = 25                        # output frames per core
BF = OUTF + 1                    # feature frames per core (1 halo)

# static pair index tables (match np.triu_indices order used by reference)
_HIU = np.triu_indices(21, 1)    # 210 hand pairs
_PIU = np.triu_indices(25, 1)    # 300 pose pairs
_LIU = np.triu_indices(20, 1)    # 190 lip pairs
NH, NP_, NL = 210, 300, 190


def _pairmat(nj, iu):
    g = np.zeros((nj, len(iu[0])), np.float32)
    g[iu[0], np.arange(len(iu[0]))] = 1.0
    g[iu[1], np.arange(len(iu[1]))] -= 1.0
    return g


def build_bass():
    nc = bacc.Bacc("TRN2", target_bir_lowering=False, debug=False,
                   num_devices=NCORES)

    hl_d = nc.dram_tensor("hl", [SHARD, HW], BF16, kind="ExternalInput")
    hr_d = nc.dram_tensor("hr", [SHARD, HW], BF16, kind="ExternalInput")
    xb_d = nc.dram_tensor("xb", [BF, 115, 3], F32, kind="ExternalInput")
    xbs_d = nc.dram_tensor("xbs", [OUTF, 115, 3], F32, kind="ExternalInput")
    # per-region joint-major layout: 5 regions x 3 coords x BF frames,
    # regions: handL, handR, pose, lip1, lip2 (each region's joints at
    # partition 0 so PE matmul base-partition rules are satisfied)
    xreg_d = nc.dram_tensor("xreg", [25, 5 * 3 * BF], F32, kind="ExternalInput")
    gh_d = nc.dram_tensor("gh", [21, NH], F32, kind="ExternalInput")
    gp_d = nc.dram_tensor("gp", [25, NP_], F32, kind="ExternalInput")
    gl_d = nc.dram_tensor("gl", [20, NL], F32, kind="ExternalInput")
    yl = nc.dram_tensor("yl", [OUTF, 1198], F32, kind="ExternalOutput")
    yr = nc.dram_tensor("yr", [OUTF, 1198], F32, kind="ExternalOutput")
    pcnt = nc.dram_tensor("pcnt", [P, 2 * NCHUNK], F32, kind="ExternalOutput")

    with tile.TileContext(nc) as tc:
        with (
            tc.tile_pool(name="cnt_in", bufs=2 * NCHUNK) as cnt_in,
            tc.tile_pool(name="cnt_scr", bufs=4) as cnt_scr,
            tc.tile_pool(name="persist", bufs=1) as persist,
            tc.tile_pool(name="fb", bufs=1) as fb,
            tc.tile_pool(name="psum", bufs=4, space=bass.MemorySpace.PSUM) as psum,
        ):
            # ---- count loads first: big contiguous stream on the SP ring ----
            hl_r = hl_d[:].rearrange("(p f) c -> p (f c)", p=P)   # [125, 6300]
            hr_r = hr_d[:].rearrange("(p f) c -> p (f c)", p=P)
            ctiles = []
            for k in range(NCHUNK):
                tl = cnt_in.tile([P, CW], BF16, tag="hl")
                nc.sync.dma_start(tl[:], hl_r[:, k * CW:(k + 1) * CW])
                tr = cnt_in.tile([P, CW], BF16, tag="hr")
                nc.sync.dma_start(tr[:], hr_r[:, k * CW:(k + 1) * CW])
                ctiles.append((tl, tr))

            # ---- feature loads on the ACT ring (parallel with count DMA) ----
            XB = fb.tile([BF, 115, 3], F32)
            nc.scalar.dma_start(XB[:], xb_d[:])
            XBs = fb.tile([OUTF, 115, 3], F32)
            nc.scalar.dma_start(XBs[:], xbs_d[:])
            XR = fb.tile([25, 5 * 3 * BF], F32)
            nc.scalar.dma_start(XR[:], xreg_d[:])
            gh = fb.tile([21, NH], F32)
            nc.scalar.dma_start(gh[:], gh_d[:])
            gp = fb.tile([25, NP_], F32)
            nc.scalar.dma_start(gp[:], gp_d[:])
            gl = fb.tile([20, NL], F32)
            nc.scalar.dma_start(gl[:], gl_d[:])

            # prime the ACT sqrt table while DMAs stream (1.3us off the path)
            prim = persist.tile([1, 2], F32)
            nc.vector.memset(prim[:], 1.0)
            nc.scalar.sqrt(prim[:, 1:2], prim[:, 0:1])

            # ---- early DVE feature ops (issued before count ops on DVE) ----
            # temporal delta (frame t - frame t+1); halo loaded from DRAM
            D = fb.tile([OUTF, 115, 3], F32)
            nc.vector.tensor_sub(D[:], XB[0:OUTF, :, :], XBs[:])

            # hand masks
            sumL = fb.tile([BF, 1], F32)
            nc.vector.reduce_sum(out=sumL[:], in_=XB[:, 40:61, :],
                                 axis=mybir.AxisListType.XY)
            sumR = fb.tile([BF, 1], F32)
            nc.vector.reduce_sum(out=sumR[:], in_=XB[:, 94:115, :],
                                 axis=mybir.AxisListType.XY)
            maskL = fb.tile([BF, 1], F32)
            nc.vector.tensor_scalar(out=maskL[:], in0=sumL[:], scalar1=0.0,
                                    scalar2=None, op0=ALU.not_equal)
            maskR = fb.tile([BF, 1], F32)
            nc.vector.tensor_scalar(out=maskR[:], in0=sumR[:], scalar1=0.0,
                                    scalar2=None, op0=ALU.not_equal)

            # ---- count compute: fused (!=0) + row-accumulate on DVE ----
            acc = persist.tile([P, 2 * NCHUNK], F32)
            for k, (tl, tr) in enumerate(ctiles):
                scr = cnt_scr.tile([P, CW], BF16, tag="scr")
                nc.vector.tensor_scalar(
                    out=scr[:], in0=tl[:], scalar1=0.0, scalar2=None,
                    op0=ALU.not_equal, op1=ALU.add,
                    accum_out=acc[:, k:k + 1])
                scr2 = cnt_scr.tile([P, CW], BF16, tag="scr2")
                nc.vector.tensor_scalar(
                    out=scr2[:], in0=tr[:], scalar1=0.0, scalar2=None,
                    op0=ALU.not_equal, op1=ALU.add,
                    accum_out=acc[:, NCHUNK + k:NCHUNK + k + 1])
            nc.sync.dma_start(pcnt[:], acc[:])

            # ---- feature phase (both variants) ----
            # mirrored-left hand coords (x negated), plain and temporal-diff
            tmpL = fb.tile([BF, 21, 3], F32)
            nc.scalar.mul(tmpL[:, :, 0:1], XB[:, 40:61, 0:1], -1.0)
            nc.scalar.copy(tmpL[:, :, 1:3], XB[:, 40:61, 1:3])
            tmpDL = fb.tile([OUTF, 21, 3], F32)
            nc.scalar.mul(tmpDL[:, :, 0:1], D[:, 40:61, 0:1], -1.0)
            nc.scalar.copy(tmpDL[:, :, 1:3], D[:, 40:61, 1:3])

            # pairwise squared distances via PE: diff_c = Xreg_c.T @ G
            def dist2(dst, region, nj, gt, npair, ncoord):
                for c in range(ncoord):
                    pdsq = psum.tile([BF, npair], F32, tag="pdif")
                    base = region * 3 * BF + c * BF
                    nc.tensor.matmul(
                        pdsq[:], XR[0:nj, base:base + BF], gt[:])
                    if c == 0:
                        nc.scalar.square(dst[:], pdsq[:])
                    else:
                        sq = fb.tile([BF, npair], F32, tag="sqt" + str(c))
                        nc.scalar.square(sq[:], pdsq[:])
                        nc.gpsimd.tensor_add(dst[:], dst[:], sq[:])

            hd2L = fb.tile([BF, NH], F32)
            dist2(hd2L, 0, 21, gh, NH, 3)
            hd2R = fb.tile([BF, NH], F32)
            dist2(hd2R, 1, 21, gh, NH, 3)
            pd2 = fb.tile([BF, NP_], F32)
            dist2(pd2, 2, 25, gp, NP_, 2)
            ol2 = fb.tile([BF, NL], F32)
            dist2(ol2, 3, 20, gl, NL, 2)
            il2 = fb.tile([BF, NL], F32)
            dist2(il2, 4, 20, gl, NL, 2)

            FEATL = fb.tile([OUTF, 1198], F32)
            FEATR = fb.tile([OUTF, 1198], F32)

            def v3(ft, lo, hi):
                return ft[:, lo:hi].rearrange("p (j c) -> p j c", c=3)

            def v2(ft, lo, hi):
                return ft[:, lo:hi].rearrange("p (j c) -> p j c", c=2)

            # right variant assembled on ACT, left variant on Pool so the
            # two copy streams run concurrently
            def act_copy(dst, src):
                nc.scalar.copy(dst, src)

            def act_add1(dst, src):
                nc.scalar.add(dst, src, 1.0)

            def pool_copy(dst, src):
                nc.gpsimd.tensor_copy(dst, src)

            def pool_add1(dst, src):
                nc.gpsimd.tensor_scalar(
                    out=dst, in0=src, scalar1=1.0, scalar2=None, op0=ALU.add)

            for cp, add1, FT, hnd, dhnd, hd2, msk in (
                    (act_copy, act_add1, FEATR, XB[0:OUTF, 94:115, :],
                     D[:, 94:115, :], hd2R, maskR),
                    (pool_copy, pool_add1, FEATL, tmpL[0:OUTF, :, :],
                     tmpDL[:], hd2L, maskL)):
                cp(v3(FT, 0, 63), hnd)
                cp(v2(FT, 63, 113), XB[0:OUTF, 61:86, 0:2])
                cp(v2(FT, 113, 153), XB[0:OUTF, 0:20, 0:2])
                cp(v3(FT, 153, 216), dhnd)
                cp(v2(FT, 216, 266), D[:, 61:86, 0:2])
                cp(v2(FT, 266, 306), D[:, 0:20, 0:2])
                cp(FT[:, 1196:1197], msk[0:OUTF, :])
                add1(FT[:, 1197:1198], msk[0:OUTF, :])

            nc.scalar.sqrt(FEATR[:, 306:516], hd2R[0:OUTF, :])
            nc.scalar.sqrt(FEATL[:, 306:516], hd2L[0:OUTF, :])
            # cond-invariant distance block: compute once, copy across
            nc.scalar.sqrt(FEATR[:, 516:816], pd2[0:OUTF, :])
            nc.scalar.sqrt(FEATR[:, 816:1006], ol2[0:OUTF, :])
            nc.scalar.sqrt(FEATR[:, 1006:1196], il2[0:OUTF, :])
            nc.gpsimd.tensor_copy(FEATL[:, 516:1196], FEATR[:, 516:1196])

            # mirror x coords of pose/lip blocks in the left variant
            for (lo, hi) in ((63, 113), (113, 153), (216, 266), (266, 306)):
                vv = v2(FEATL, lo, hi)
                nc.vector.tensor_scalar(
                    out=vv[:, :, 0:1], in0=vv[:, :, 0:1], scalar1=-1.0,
                    scalar2=None, op0=ALU.mult)

            nc.scalar.dma_start(yr[:], FEATR[:])
            nc.scalar.dma_start(yl[:], FEATL[:])

    nc.compile()
    return nc


_NC_CACHE = None


def _get_nc():
    global _NC_CACHE
    if _NC_CACHE is None:
        _NC_CACHE = build_bass()
    return _NC_CACHE


def make_in_maps(x: np.ndarray):
    x = np.ascontiguousarray(np.asarray(x, dtype=np.float32))
    assert x.shape == (T_TOT, 115, 3)
    # contiguous bf16 hand blocks for the count phase (exact: bf16 rounds
    # to zero only below 2^-133; any normal f32 nonzero stays nonzero)
    lh = x[:, 40:61, :].astype(ml_dtypes.bfloat16).reshape(T_TOT, HW)
    rh = x[:, 94:115, :].astype(ml_dtypes.bfloat16).reshape(T_TOT, HW)
    gh = _pairmat(21, _HIU)
    gp = _pairmat(25, _PIU)
    gl = _pairmat(20, _LIU)
    in_maps = []
    regions = ((40, 61), (94, 115), (61, 86), (0, 20), (20, 40))
    for c in range(NCORES):
        xb = x[c * OUTF:c * OUTF + BF]                      # [26,115,3]
        xbs = x[c * OUTF + 1:c * OUTF + BF]                 # [25,115,3]
        xreg = np.zeros((25, 5 * 3 * BF), np.float32)
        for r, (j0, j1) in enumerate(regions):
            blk = xb[:, j0:j1, :].transpose(1, 2, 0)        # [J,3,BF]
            xreg[0:j1 - j0, r * 3 * BF:(r + 1) * 3 * BF] = \
                blk.reshape(j1 - j0, 3 * BF)
        in_maps.append({
            "hl": lh[c * SHARD:(c + 1) * SHARD],
            "hr": rh[c * SHARD:(c + 1) * SHARD],
            "xb": np.ascontiguousarray(xb),
            "xbs": np.ascontiguousarray(xbs),
            "xreg": xreg,
            "gh": gh, "gp": gp, "gl": gl,
        })
    return in_maps


def run_device(x: np.ndarray, **kw):
    nc = _get_nc()
    in_maps = make_in_maps(x)
    res = bass_utils.run_bass_kernel_spmd(
        nc, in_maps, core_ids=list(range(NCORES)), **kw)
    # global left/right decision from the exact integer-valued partials
    diff = 0.0
    for r in res.results:
        pc = np.asarray(r["pcnt"], dtype=np.float64)
        diff += pc[:, :NCHUNK].sum() - pc[:, NCHUNK:].sum()
    key = "yl" if diff > 0 else "yr"
    out = np.concatenate([r[key] for r in res.results], axis=0)
    return out.reshape(1, 200, 1198).astype(np.float32, copy=False), res


def kernel(x: np.ndarray) -> np.ndarray:
    return run_device(x)[0]


if __name__ == "__main__":
    rng = np.random.default_rng(0)
    x = rng.standard_normal((T_TOT, 115, 3), dtype=np.float32)
    out = kernel(x)
    print(out.shape, out.dtype, float(np.linalg.norm(out)))


# revision 4
# speedup vs baseline: 2.3315x; 1.1224x over previous
"""Trainium2 Bass kernel for nn_FeatureGenKerasV2.

Contract: kernel(x) with x [100000, 115, 3] f32 -> [1, 200, 1198] f32.

Reference semantics:
  - global: cond = (count_nonzero(x[:,40:61]) > count_nonzero(x[:,94:115]))
  - per frame t<200: features built from hand(sel by cond)/pose/lip coords,
    temporal diff vs frame t+1, static-pair distances, hand mask.

Sharding (8 cores, embarrassingly parallel over frames):
  - count phase: core c counts nonzeros of both hand regions over frames
    [12500c, 12500(c+1)). The hand columns are sliced out host-side into
    contiguous per-core [128, 6300] bf16 arrays (zero-padded; bf16 is
    exactly count-preserving for |x| >= 2^-133) so the device streams them
    at DMA line rate across all 16 SDMA engines. The per-partition
    per-chunk partial sums [128, 2*NCHUNK] are returned raw and the host
    performs the exact scalar all-reduce (integer-valued f32, summed f64).
  - feature phase: core c computes BOTH left/right feature variants for its
    output frames [25c, 25c+25) (1-frame halo sliced host-side) and writes
    yl_c/yr_c [25, 1198].
  - unshard: host sums the partials, picks the variant (cond = diff > 0),
    and concatenates the per-core slices.
"""

import numpy as np
import ml_dtypes

import concourse.bass as bass
import concourse.tile as tile
from concourse import bacc, mybir
from concourse import bass_utils

F32 = mybir.dt.float32
BF16 = mybir.dt.bfloat16
ALU = mybir.AluOpType

NCORES = 8
T_TOT = 100000
SHARD = T_TOT // NCORES          # 12500 count frames per core
HW = 63                          # hand elements per frame
P = 128                          # SBUF partitions for counting
EPP = 6300                       # padded elems per partition (128*6300 >= 12500*63)
NCHUNK = 5                       # count chunks per hand
CW = EPP // NCHUNK               # 1260 elems per partition per chunk
COUNT_STT = True                 # scalar_tensor_tensor vs tensor_scalar count
OUTF = 25                        # output frames per core
BF = OUTF + 1                    # feature frames per core (1 halo)

# packed feature-input column offsets: xb | xbs | xreg | gh | gp | gl
FXW = 345 + 345 + 390 + 210 + 300 + 190   # 1780

# static pair index tables (match np.triu_indices order used by reference)
_HIU = np.triu_indices(21, 1)    # 210 hand pairs
_PIU = np.triu_indices(25, 1)    # 300 pose pairs
_LIU = np.triu_indices(20, 1)    # 190 lip pairs
NH, NP_, NL = 210, 300, 190


def _pairmat(nj, iu):
    g = np.zeros((nj, len(iu[0])), np.float32)
    g[iu[0], np.arange(len(iu[0]))] = 1.0
    g[iu[1], np.arange(len(iu[1]))] -= 1.0
    return g


def build_bass():
    nc = bacc.Bacc("TRN2", target_bir_lowering=False, debug=False,
                   num_devices=NCORES)

    hl_d = nc.dram_tensor("hl", [P, EPP], BF16, kind="ExternalInput")
    hr_d = nc.dram_tensor("hr", [P, EPP], BF16, kind="ExternalInput")
    fx_d = nc.dram_tensor("fx", [BF, FXW], F32, kind="ExternalInput")
    yl = nc.dram_tensor("yl", [OUTF, 1198], F32, kind="ExternalOutput")
    yr = nc.dram_tensor("yr", [OUTF, 1198], F32, kind="ExternalOutput")
    pcnt = nc.dram_tensor("pcnt", [P, 2 * NCHUNK], F32, kind="ExternalOutput")

    with tile.TileContext(nc) as tc:
        with (
            tc.tile_pool(name="cnt_in", bufs=2 * NCHUNK) as cnt_in,
            tc.tile_pool(name="cnt_scr", bufs=4) as cnt_scr,
            tc.tile_pool(name="persist", bufs=1) as persist,
            tc.tile_pool(name="fb", bufs=1) as fb,
            tc.tile_pool(name="psum", bufs=4, space=bass.MemorySpace.PSUM) as psum,
        ):
            # ---- count loads first: big contiguous stream on the SP ring ----
            ctiles = []
            for k in range(NCHUNK):
                tl = cnt_in.tile([P, CW], BF16, tag="hl")
                nc.sync.dma_start(tl[:], hl_d[:, k * CW:(k + 1) * CW])
                tr = cnt_in.tile([P, CW], BF16, tag="hr")
                nc.sync.dma_start(tr[:], hr_d[:, k * CW:(k + 1) * CW])
                ctiles.append((tl, tr))

            # ---- one packed feature load on the ACT ring ----
            FX = fb.tile([BF, FXW], F32)
            nc.scalar.dma_start(FX[:], fx_d[:])
            XB = FX[:, 0:345].rearrange("p (j c) -> p j c", c=3)
            XBs = FX[0:OUTF, 345:690].rearrange("p (j c) -> p j c", c=3)
            XR = FX[0:25, 690:1080]
            gh = FX[0:21, 1080:1290]
            gp = FX[0:25, 1290:1590]
            gl = FX[0:20, 1590:1780]

            # prime the ACT sqrt table while DMAs stream (1.3us off the path)
            prim = persist.tile([1, 2], F32)
            nc.gpsimd.memset(prim[:], 1.0)
            nc.scalar.sqrt(prim[:, 1:2], prim[:, 0:1])

            # zeros operand for the packed scalar_tensor_tensor count
            zt = persist.tile([P, CW], BF16)
            nc.gpsimd.memset(zt[:], 0.0)

            # ---- early DVE feature ops (issued before count ops on DVE) ----
            # temporal delta (frame t - frame t+1); halo column-packed in fx
            D = fb.tile([OUTF, 115, 3], F32)
            nc.vector.tensor_sub(D[:], XB[0:OUTF, :, :], XBs[:])

            # hand masks
            sumL = fb.tile([BF, 1], F32)
            nc.vector.reduce_sum(out=sumL[:], in_=XB[:, 40:61, :],
                                 axis=mybir.AxisListType.XY)
            sumR = fb.tile([BF, 1], F32)
            nc.vector.reduce_sum(out=sumR[:], in_=XB[:, 94:115, :],
                                 axis=mybir.AxisListType.XY)
            maskL = fb.tile([BF, 1], F32)
            nc.vector.tensor_scalar(out=maskL[:], in0=sumL[:], scalar1=0.0,
                                    scalar2=None, op0=ALU.not_equal)
            maskR = fb.tile([BF, 1], F32)
            nc.vector.tensor_scalar(out=maskR[:], in0=sumR[:], scalar1=0.0,
                                    scalar2=None, op0=ALU.not_equal)

            # ---- count compute: fused (!=0) + row-accumulate on DVE ----
            acc = persist.tile([P, 2 * NCHUNK], F32)
            for k, (tl, tr) in enumerate(ctiles):
                for i, tt in enumerate((tl, tr)):
                    scr = cnt_scr.tile([P, CW], BF16, tag="scr" + str(i))
                    ac = acc[:, i * NCHUNK + k:i * NCHUNK + k + 1]
                    if COUNT_STT:
                        nc.vector.scalar_tensor_tensor(
                            out=scr[:], in0=tt[:], scalar=0.0, in1=zt[:],
                            op0=ALU.bypass, op1=ALU.not_equal,
                            accum_out=ac)
                    else:
                        nc.vector.tensor_scalar(
                            out=scr[:], in0=tt[:], scalar1=0.0, scalar2=None,
                            op0=ALU.not_equal, op1=ALU.add, accum_out=ac)
            nc.sync.dma_start(pcnt[:], acc[:])

            # ---- feature phase (both variants) ----
            # pairwise squared distances via PE: diff_c = Xreg_c.T @ G
            def dist2(dst, region, nj, gt, npair, ncoord):
                for c in range(ncoord):
                    pdsq = psum.tile([BF, npair], F32, tag="pdif")
                    base = region * 3 * BF + c * BF
                    nc.tensor.matmul(
                        pdsq[:], XR[0:nj, base:base + BF], gt[:])
                    if c == 0:
                        nc.scalar.square(dst[:], pdsq[:])
                    else:
                        sq = fb.tile([BF, npair], F32, tag="sqt" + str(c))
                        nc.scalar.square(sq[:], pdsq[:])
                        nc.gpsimd.tensor_add(dst[:], dst[:], sq[:])

            hd2L = fb.tile([BF, NH], F32)
            dist2(hd2L, 0, 21, gh, NH, 3)
            hd2R = fb.tile([BF, NH], F32)
            dist2(hd2R, 1, 21, gh, NH, 3)
            pd2 = fb.tile([BF, NP_], F32)
            dist2(pd2, 2, 25, gp, NP_, 2)
            ol2 = fb.tile([BF, NL], F32)
            dist2(ol2, 3, 20, gl, NL, 2)
            il2 = fb.tile([BF, NL], F32)
            dist2(il2, 4, 20, gl, NL, 2)

            FEATL = fb.tile([OUTF, 1198], F32)
            FEATR = fb.tile([OUTF, 1198], F32)

            def v3(ft, lo, hi):
                return ft[:, lo:hi].rearrange("p (j c) -> p j c", c=3)

            def v2(ft, lo, hi):
                return ft[:, lo:hi].rearrange("p (j c) -> p j c", c=2)

            # right variant assembled on ACT, left variant on Pool so the
            # two copy streams run concurrently; left-hand mirroring is done
            # in place afterwards with the other x-mirror ops
            def act_copy(dst, src):
                nc.scalar.copy(dst, src)

            def act_add1(dst, src):
                nc.scalar.add(dst, src, 1.0)

            def pool_copy(dst, src):
                nc.gpsimd.tensor_copy(dst, src)

            def pool_add1(dst, src):
                nc.gpsimd.tensor_scalar(
                    out=dst, in0=src, scalar1=1.0, scalar2=None, op0=ALU.add)

            for cp, add1, FT, jh in (
                    (act_copy, act_add1, FEATR, (94, 115)),
                    (pool_copy, pool_add1, FEATL, (40, 61))):
                cp(v3(FT, 0, 63), XB[0:OUTF, jh[0]:jh[1], :])
                cp(v2(FT, 63, 113), XB[0:OUTF, 61:86, 0:2])
                cp(v2(FT, 113, 153), XB[0:OUTF, 0:20, 0:2])
                cp(v3(FT, 153, 216), D[:, jh[0]:jh[1], :])
                cp(v2(FT, 216, 266), D[:, 61:86, 0:2])
                cp(v2(FT, 266, 306), D[:, 0:20, 0:2])
            act_copy(FEATR[:, 1196:1197], maskR[0:OUTF, :])
            act_add1(FEATR[:, 1197:1198], maskR[0:OUTF, :])
            pool_copy(FEATL[:, 1196:1197], maskL[0:OUTF, :])
            pool_add1(FEATL[:, 1197:1198], maskL[0:OUTF, :])

            nc.scalar.sqrt(FEATR[:, 306:516], hd2R[0:OUTF, :])
            nc.scalar.sqrt(FEATL[:, 306:516], hd2L[0:OUTF, :])
            # cond-invariant distance block: compute once, write to both
            for (lo, hi), t2 in (((516, 816), pd2), ((816, 1006), ol2),
                                 ((1006, 1196), il2)):
                nc.scalar.sqrt(FEATR[:, lo:hi], t2[0:OUTF, :])
                nc.gpsimd.tensor_copy(FEATL[:, lo:hi], FEATR[:, lo:hi])

            # mirror x coords in the left variant (hand blocks incl.)
            for (lo, hi), w in (((0, 63), 3), ((63, 113), 2), ((113, 153), 2),
                                ((153, 216), 3), ((216, 266), 2),
                                ((266, 306), 2)):
                vv = FEATL[:, lo:hi].rearrange("p (j c) -> p j c", c=w)
                nc.vector.tensor_scalar(
                    out=vv[:, :, 0:1], in0=vv[:, :, 0:1], scalar1=-1.0,
                    scalar2=None, op0=ALU.mult)

            nc.scalar.dma_start(yr[:], FEATR[:])
            nc.scalar.dma_start(yl[:], FEATL[:])

    nc.compile()
    return nc


_NC_CACHE = None


def _get_nc():
    global _NC_CACHE
    if _NC_CACHE is None:
        _NC_CACHE = build_bass()
    return _NC_CACHE


def make_in_maps(x: np.ndarray):
    x = np.ascontiguousarray(np.asarray(x, dtype=np.float32))
    assert x.shape == (T_TOT, 115, 3)
    # contiguous bf16 hand blocks for the count phase (exact: bf16 rounds
    # to zero only below 2^-133; any nonzero input value stays nonzero)
    lh = x[:, 40:61, :].astype(ml_dtypes.bfloat16).reshape(T_TOT, HW)
    rh = x[:, 94:115, :].astype(ml_dtypes.bfloat16).reshape(T_TOT, HW)
    gh = _pairmat(21, _HIU)
    gp = _pairmat(25, _PIU)
    gl = _pairmat(20, _LIU)
    in_maps = []
    regions = ((40, 61), (94, 115), (61, 86), (0, 20), (20, 40))
    for c in range(NCORES):
        hlp = np.zeros((P, EPP), ml_dtypes.bfloat16)
        hrp = np.zeros((P, EPP), ml_dtypes.bfloat16)
        hlp.reshape(-1)[:SHARD * HW] = lh[c * SHARD:(c + 1) * SHARD].reshape(-1)
        hrp.reshape(-1)[:SHARD * HW] = rh[c * SHARD:(c + 1) * SHARD].reshape(-1)
        xb = x[c * OUTF:c * OUTF + BF]                      # [26,115,3]
        fx = np.zeros((BF, FXW), np.float32)
        fx[:, 0:345] = xb.reshape(BF, 345)
        fx[0:OUTF, 345:690] = x[c * OUTF + 1:c * OUTF + BF].reshape(OUTF, 345)
        for r, (j0, j1) in enumerate(regions):
            blk = xb[:, j0:j1, :].transpose(1, 2, 0)        # [J,3,BF]
            fx[0:j1 - j0, 690 + r * 3 * BF:690 + (r + 1) * 3 * BF] = \
                blk.reshape(j1 - j0, 3 * BF)
        fx[0:21, 1080:1290] = gh
        fx[0:25, 1290:1590] = gp
        fx[0:20, 1590:1780] = gl
        in_maps.append({"hl": hlp, "hr": hrp, "fx": fx})
    return in_maps


def run_device(x: np.ndarray, **kw):
    nc = _get_nc()
    in_maps = make_in_maps(x)
    res = bass_utils.run_bass_kernel_spmd(
        nc, in_maps, core_ids=list(range(NCORES)), **kw)
    # global left/right decision from the exact integer-valued partials
    diff = 0.0
    for r in res.results:
        pc = np.asarray(r["pcnt"], dtype=np.float64)
        diff += pc[:, :NCHUNK].sum() - pc[:, NCHUNK:].sum()
    key = "yl" if diff > 0 else "yr"
    out = np.concatenate([r[key] for r in res.results], axis=0)
    return out.reshape(1, 200, 1198).astype(np.float32, copy=False), res


def kernel(x: np.ndarray) -> np.ndarray:
    return run_device(x)[0]


if __name__ == "__main__":
    rng = np.random.default_rng(0)
    x = rng.standard_normal((T_TOT, 115, 3), dtype=np.float32)
    out = kernel(x)
    print(out.shape, out.dtype, float(np.linalg.norm(out)))


# revision 19
# speedup vs baseline: 2.4219x; 1.0388x over previous
"""Trainium2 Bass kernel for nn_FeatureGenKerasV2.

Contract: kernel(x) with x [100000, 115, 3] f32 -> [1, 200, 1198] f32.

Reference semantics:
  - global: cond = (count_nonzero(x[:,40:61]) > count_nonzero(x[:,94:115]))
  - per frame t<200: features built from hand(sel by cond)/pose/lip coords,
    temporal diff vs frame t+1, static-pair distances, hand mask.

Sharding (8 cores, embarrassingly parallel over frames):
  - count phase: core c counts nonzeros of both hand regions over frames
    [12500c, 12500(c+1)). The hand columns are sliced out host-side into
    contiguous per-core [128, 6300] bf16 arrays (zero-padded; bf16 is
    exactly count-preserving for |x| >= 2^-133) so the device streams them
    at DMA line rate. Counting is split across DVE (fused is_ne+accum),
    and Pool-indicators summed by ACT Copy+accum; per-partition partial
    sums are returned raw and the host performs the exact scalar
    all-reduce (integer-valued f32, summed in f64).
  - feature phase: core c computes BOTH left/right feature variants for its
    output frames [25c, 25c+25) (1-frame halo sliced host-side). The
    static feature columns (hand/pose/lip coords, pre-mirrored for the
    left variant) are packed host-side and DMA'd straight into the output
    tiles; temporal diffs are one dense subtract per variant; pair
    distances run as PE matmuls packed into PSUM quadrants with a single
    square/add/sqrt pass, then land via partition-shifting SBUF DMAs.
  - unshard: host sums the partials, picks the variant (cond = diff > 0),
    and concatenates the per-core slices.
"""

import numpy as np
import ml_dtypes

import concourse.bass as bass
import concourse.tile as tile
from concourse import bacc, mybir
from concourse import bass_utils

F32 = mybir.dt.float32
BF16 = mybir.dt.bfloat16
ALU = mybir.AluOpType
ACTF = mybir.ActivationFunctionType

NCORES = 8
T_TOT = 100000
SHARD = T_TOT // NCORES          # 12500 count frames per core
HW = 63                          # hand elements per frame
P = 128                          # SBUF partitions for counting
EPP = 6300                       # padded elems per partition (128*6300 >= 12500*63)
CWS = (1575, 1575, 1575, 945, 630)   # uneven chunks: small tail
NCHUNK = len(CWS)
DMA_LAST = 315                   # descriptor split to spread SDMA engines
OUTF = 25                        # output frames per core
BF = OUTF + 1                    # feature frames per core (1 halo)

# packed feature-input column offsets (fx [26, 1828]):
#   xfR | xfRs | xfL | xfLs | hands | xreg | gh | gp | gl
OXR, OXRS, OXL, OXLS = 0, 153, 306, 459
OHND, OXREG, OGH, OGP, OGL = 612, 738, 1128, 1338, 1638
FXW = 1828

_HIU = np.triu_indices(21, 1)    # 210 hand pairs
_PIU = np.triu_indices(25, 1)    # 300 pose pairs
_LIU = np.triu_indices(20, 1)    # 190 lip pairs
NH, NP_, NL = 210, 300, 190
DW = 402                         # packed distance tile width (210 | 2 | 190)


def _pairmat(nj, iu):
    g = np.zeros((nj, len(iu[0])), np.float32)
    g[iu[0], np.arange(len(iu[0]))] = 1.0
    g[iu[1], np.arange(len(iu[1]))] -= 1.0
    return g


def build_bass():
    nc = bacc.Bacc("TRN2", target_bir_lowering=False, debug=False,
                   num_devices=NCORES)

    hl_d = nc.dram_tensor("hl", [P, EPP], BF16, kind="ExternalInput")
    hr_d = nc.dram_tensor("hr", [P, EPP], BF16, kind="ExternalInput")
    fx_d = nc.dram_tensor("fx", [BF, FXW], F32, kind="ExternalInput")
    yl = nc.dram_tensor("yl", [OUTF, 1198], F32, kind="ExternalOutput")
    yr = nc.dram_tensor("yr", [OUTF, 1198], F32, kind="ExternalOutput")
    pcl = nc.dram_tensor("pcl", [P, NCHUNK], F32, kind="ExternalOutput")
    pcr = nc.dram_tensor("pcr", [P, NCHUNK], F32, kind="ExternalOutput")

    with tile.TileContext(nc) as tc:
        with (
            tc.tile_pool(name="cnt_in", bufs=1) as cnt_in,
            tc.tile_pool(name="cnt_scr", bufs=1) as cnt_scr,
            tc.tile_pool(name="persist", bufs=1) as persist,
            tc.tile_pool(name="fb", bufs=1) as fb,
            tc.tile_pool(name="psum", bufs=1, space=bass.MemorySpace.PSUM) as psum,
        ):
            # ---- count loads first: contiguous stream on the SP ring,
            # forced-small descriptors to spread across more SDMA engines ----
            ctiles = []
            off = 0
            for k, cw in enumerate(CWS):
                tl = cnt_in.tile([P, cw], BF16, tag="hl" + str(k))
                nc.sync.dma_start(tl[:], hl_d[:, off:off + cw])
                tr = cnt_in.tile([P, cw], BF16, tag="hr" + str(k))
                nc.sync.dma_start(tr[:], hr_d[:, off:off + cw])
                ctiles.append((tl, tr))
                off += cw

            # ---- feature loads on the ACT ring: one packed tensor, plus
            # the static feature head columns straight into the outputs ----
            FX = fb.tile([BF, FXW], F32)
            nc.scalar.dma_start(FX[:], fx_d[:])
            FEATL = fb.tile([OUTF, 1198], F32)
            FEATR = fb.tile([OUTF, 1198], F32)
            nc.scalar.dma_start(FEATR[:, 0:153], fx_d[0:OUTF, OXR:OXR + 153])
            nc.scalar.dma_start(FEATL[:, 0:153], fx_d[0:OUTF, OXL:OXL + 153])

            # prime the ACT sqrt table while DMAs stream (1.3us off the path)
            prim = persist.tile([1, 3], F32)
            nc.gpsimd.memset(prim[:], 1.0)
            nc.scalar.sqrt(prim[:, 1:2], prim[:, 0:1])

            # zeros operand for the tensor_tensor_reduce count variant
            zt = persist.tile([P, CWS[0]], BF16)
            nc.gpsimd.memset(zt[:], 0.0)

            # ---- early DVE feature ops (issued before count ops on DVE) ----
            # temporal deltas: one dense subtract per variant, straight into
            # the output feature columns
            nc.vector.tensor_sub(FEATR[:, 153:306], FX[0:OUTF, OXR:OXR + 153],
                                 FX[0:OUTF, OXRS:OXRS + 153])
            nc.vector.tensor_sub(FEATL[:, 153:306], FX[0:OUTF, OXL:OXL + 153],
                                 FX[0:OUTF, OXLS:OXLS + 153])

            # hand masks
            sumL = fb.tile([BF, 1], F32)
            nc.vector.reduce_sum(out=sumL[:], in_=FX[:, OHND:OHND + 63],
                                 axis=mybir.AxisListType.X)
            sumR = fb.tile([BF, 1], F32)
            nc.vector.reduce_sum(out=sumR[:], in_=FX[:, OHND + 63:OHND + 126],
                                 axis=mybir.AxisListType.X)
            maskL = fb.tile([BF, 1], F32)
            nc.vector.tensor_scalar(out=maskL[:], in0=sumL[:], scalar1=0.0,
                                    scalar2=None, op0=ALU.not_equal)
            maskR = fb.tile([BF, 1], F32)
            nc.vector.tensor_scalar(out=maskR[:], in0=sumR[:], scalar1=0.0,
                                    scalar2=None, op0=ALU.not_equal)

            # ---- count compute ----
            # DVE: all L chunks (L0 via tensor_tensor_reduce as a packed-mode
            # probe) + R0. Pool is_ne indicators + ACT Copy-accum: R1..R4
            # (ACT accums interleaved below by expected data arrival).
            accL = persist.tile([P, NCHUNK], F32)
            accR = persist.tile([P, NCHUNK], F32)
            ind = {}
            for k, (tl, tr) in enumerate(ctiles):
                cw = CWS[k]
                scr = cnt_scr.tile([P, cw], BF16, tag="scrL" + str(k))
                nc.vector.tensor_scalar(
                    out=scr[:], in0=tl[:], scalar1=0.0, scalar2=None,
                    op0=ALU.not_equal, op1=ALU.add,
                    accum_out=accL[:, k:k + 1])
                scr2 = cnt_scr.tile([P, cw], BF16, tag="scrR" + str(k))
                nc.vector.tensor_scalar(
                    out=scr2[:], in0=tr[:], scalar1=0.0, scalar2=None,
                    op0=ALU.not_equal, op1=ALU.add,
                    accum_out=accR[:, k:k + 1])
            nc.sync.dma_start(pcl[:], accL[:])

            # ---- masks into the feature tails (ACT, gated early) ----
            nc.scalar.copy(FEATR[:, 1196:1197], maskR[0:OUTF, :])
            nc.scalar.add(FEATR[:, 1197:1198], maskR[0:OUTF, :], 1.0)
            nc.scalar.copy(FEATL[:, 1196:1197], maskL[0:OUTF, :])
            nc.scalar.add(FEATL[:, 1197:1198], maskL[0:OUTF, :], 1.0)

            # ---- pairwise squared distances via PE, packed into PSUM
            # quadrants (base partitions limited to 0/32/64):
            #   handL @ partitions 0:26  cols 0:210, outer lips cols 212:402
            #   handR @ partitions 32:58 cols 0:210, inner lips cols 212:402
            #   pose  @ partitions 64:90 cols 0:300
            # one square/add/sqrt pass handles every region at once ----
            PP = 96
            PD0 = psum.tile([PP, DW], F32, tag="pd0")
            PD1 = psum.tile([PP, DW], F32, tag="pd1")
            PD2 = psum.tile([64, 212], F32, tag="pd2")
            gh = FX[0:21, OGH:OGH + NH]
            gp = FX[0:25, OGP:OGP + NP_]
            gl = FX[0:20, OGL:OGL + NL]
            blocks = (  # (region, nj, G, npair, part_off, col_off, ncoord)
                (0, 21, gh, NH, 0, 0, 3),
                (1, 21, gh, NH, 32, 0, 3),
                (2, 25, gp, NP_, 64, 0, 2),
                (3, 20, gl, NL, 0, 212, 2),
                (4, 20, gl, NL, 32, 212, 2),
            )
            for c, PD in ((0, PD0), (1, PD1), (2, PD2)):
                for (r, nj, gt, npair, po, co, ncoord) in blocks:
                    if c >= ncoord:
                        continue
                    base = OXREG + r * 3 * BF + c * BF
                    nc.tensor.matmul(
                        PD[po:po + BF, co:co + npair],
                        FX[0:nj, base:base + BF], gt)
            SQ0 = fb.tile([PP, DW], F32)
            nc.scalar.square(SQ0[:], PD0[:])
            SQ1 = fb.tile([PP, DW], F32)
            nc.scalar.square(SQ1[:], PD1[:])
            SQ2 = fb.tile([64, 212], F32)
            nc.scalar.square(SQ2[:], PD2[:])

            # coord sums on Pool (full-width tiles use 6 of 8 Q7 cores)
            S = fb.tile([PP, DW], F32)
            nc.gpsimd.tensor_add(S[:], SQ0[:], SQ1[:])
            nc.gpsimd.tensor_add(S[0:64, 0:212], S[0:64, 0:212], SQ2[:])
            SD = fb.tile([PP, DW], F32)
            nc.scalar.sqrt(SD[:], S[:])

            # distance blocks land in the feature rows via partition-shifting
            # SBUF->SBUF DMAs (FEATR on the ACT ring, FEATL on the SP ring)
            for eng, FT, hoff in ((nc.scalar, FEATR, 32), (nc.sync, FEATL, 0)):
                eng.dma_start(FT[:, 306:516], SD[hoff:hoff + OUTF, 0:210])
                eng.dma_start(FT[:, 516:816], SD[64:64 + OUTF, 0:300])
                eng.dma_start(FT[:, 816:1006], SD[0:OUTF, 212:402])
                eng.dma_start(FT[:, 1006:1196], SD[32:32 + OUTF, 212:402])

            nc.scalar.dma_start(pcr[:], accR[:])

            nc.scalar.dma_start(yr[:], FEATR[:])
            nc.scalar.dma_start(yl[:], FEATL[:])

    nc.compile()
    return nc


_NC_CACHE = None


def _get_nc():
    global _NC_CACHE
    if _NC_CACHE is None:
        _NC_CACHE = build_bass()
    return _NC_CACHE


def make_in_maps(x: np.ndarray):
    x = np.ascontiguousarray(np.asarray(x, dtype=np.float32))
    assert x.shape == (T_TOT, 115, 3)
    # contiguous bf16 hand blocks for the count phase (exact: bf16 rounds
    # to zero only below 2^-133; any nonzero input value stays nonzero)
    lh = x[:, 40:61, :].astype(ml_dtypes.bfloat16).reshape(T_TOT, HW)
    rh = x[:, 94:115, :].astype(ml_dtypes.bfloat16).reshape(T_TOT, HW)
    gh = _pairmat(21, _HIU)
    gp = _pairmat(25, _PIU)
    gl = _pairmat(20, _LIU)
    in_maps = []
    regions = ((40, 61), (94, 115), (61, 86), (0, 20), (20, 40))

    def xfeat(xb, jh, mir):
        h = xb[:, jh[0]:jh[1], :].reshape(BF, 63).copy()
        p = xb[:, 61:86, 0:2].reshape(BF, 50).copy()
        l = xb[:, 0:20, 0:2].reshape(BF, 40).copy()
        if mir:
            h[:, 0::3] *= -1.0
            p[:, 0::2] *= -1.0
            l[:, 0::2] *= -1.0
        return np.concatenate([h, p, l], axis=1)    # [26,153]

    for c in range(NCORES):
        hlp = np.zeros((P, EPP), ml_dtypes.bfloat16)
        hrp = np.zeros((P, EPP), ml_dtypes.bfloat16)
        hlp.reshape(-1)[:SHARD * HW] = lh[c * SHARD:(c + 1) * SHARD].reshape(-1)
        hrp.reshape(-1)[:SHARD * HW] = rh[c * SHARD:(c + 1) * SHARD].reshape(-1)
        xb = x[c * OUTF:c * OUTF + BF]                      # [26,115,3]
        fx = np.zeros((BF, FXW), np.float32)
        xfR = xfeat(xb, (94, 115), False)
        xfL = xfeat(xb, (40, 61), True)
        fx[:, OXR:OXR + 153] = xfR
        fx[0:OUTF, OXRS:OXRS + 153] = xfR[1:BF]
        fx[:, OXL:OXL + 153] = xfL
        fx[0:OUTF, OXLS:OXLS + 153] = xfL[1:BF]
        fx[:, OHND:OHND + 63] = xb[:, 40:61, :].reshape(BF, 63)
        fx[:, OHND + 63:OHND + 126] = xb[:, 94:115, :].reshape(BF, 63)
        for r, (j0, j1) in enumerate(regions):
            blk = xb[:, j0:j1, :].transpose(1, 2, 0)        # [J,3,BF]
            fx[0:j1 - j0, OXREG + r * 3 * BF:OXREG + (r + 1) * 3 * BF] = \
                blk.reshape(j1 - j0, 3 * BF)
        fx[0:21, OGH:OGH + NH] = gh
        fx[0:25, OGP:OGP + NP_] = gp
        fx[0:20, OGL:OGL + NL] = gl
        in_maps.append({"hl": hlp, "hr": hrp, "fx": fx})
    return in_maps


def run_device(x: np.ndarray, **kw):
    nc = _get_nc()
    in_maps = make_in_maps(x)
    res = bass_utils.run_bass_kernel_spmd(
        nc, in_maps, core_ids=list(range(NCORES)), **kw)
    # global left/right decision from the exact integer-valued partials
    diff = 0.0
    for r in res.results:
        diff += (np.asarray(r["pcl"], dtype=np.float64).sum()
                 - np.asarray(r["pcr"], dtype=np.float64).sum())
    key = "yl" if diff > 0 else "yr"
    out = np.concatenate([r[key] for r in res.results], axis=0)
    return out.reshape(1, 200, 1198).astype(np.float32, copy=False), res


def kernel(x: np.ndarray) -> np.ndarray:
    return run_device(x)[0]


if __name__ == "__main__":
    rng = np.random.default_rng(0)
    x = rng.standard_normal((T_TOT, 115, 3), dtype=np.float32)
    out = kernel(x)
    print(out.shape, out.dtype, float(np.linalg.norm(out)))
